# revision 11
# baseline (speedup 1.0000x reference)
"""BCP quantized linear SPMD kernel for 8 Trainium2 NeuronCores.

Computes y = x @ W_deq.T + bias where
  W_deq = ((W_q - zeros) * scales) * mu2[:,None] * mu1[None,:] * mask

Sharding: tensor-parallel along the output dim K (8192 -> 1024 rows/core).
x and mu1 are replicated; the [64, 1024] per-core outputs are concatenated
on the host.

v5 dataflow: the host re-encodes the int4+zero-point+mask weights as
symmetric per-group int8:
    W8[k,n] = round(127 * (q - z) / alpha) * mask,
    alpha[k,g] = max|q - z| over the group (masked),
with alpha/127, mu2 and the quant scales folded into one per-group f16
scale tensor (uploaded pre-pair-duplicated for broadcast APs), and mu1
folded into a pre-transposed f16 x upload.  Under a symmetric encoding
the pruned weights are exactly 0, so the mask and the zero-point
subtraction vanish from the device inner loop:

  - one int8 stream cast-DMA'd to f16 SBUF tiles (k on partitions),
  - dequant = ONE pair-broadcast tensor_tensor (w = W8 * s4) per tile,
  - PE transposes [128,128] blocks in is_transpose mode (f16 PSUM out),
    16 blocks per [128, 2048] PSUM gather tile; evacuation alternates
    ScalarE / DVE; y[64, k] += xT.T @ wT accumulates per 512-wide
    k-superblock with the bias preloaded via a ones x bias matmul.
"""
import numpy as np

import concourse.bacc as bacc
import concourse.mybir as mybir
from concourse.tile import TileContext
from concourse import bass_utils

M = 64        # tokens
N = 8192      # in features
K = 8192      # out features
GS = 64       # quant group size
NG = N // GS  # 128 groups
N_CORES = 8
KL = K // N_CORES   # 1024 out rows per core
NKT = KL // 128     # 8 k tiles per core
NT = N // 128       # 64 n tiles
F16 = mybir.dt.float16
F32 = mybir.dt.float32
I8 = mybir.dt.int8

# n-phase widths per k-superblock (ks): small phases at the kernel's head
# and tail shrink pipeline fill/drain; large in the middle for DMA/DVE
# instruction efficiency.
WIDTHS0 = [512, 512, 1024, 2048, 4096]
WIDTHS1 = [4096, 2048, 1024, 512, 512]

_compiled = None


def _build():
    nc = bacc.Bacc("TRN2", target_bir_lowering=False)

    # W8 stream is pre-tiled on the host: one contiguous [128, nw] slab
    # per (ks, phase, k4) in emission order.
    d_b = nc.declare_dram_parameter("b", [128, NKT * N], I8, isOutput=False)
    # pair-duplicated per-(k,group) scales, host-folded:
    # s4d[p, kt*2NG + 2g + t] = scales*mu2*alpha/127
    d_s4d = nc.declare_dram_parameter("s4d", [128, NKT * NG * 2], F16, isOutput=False)
    d_bias = nc.declare_dram_parameter("bias", [1, KL], F16, isOutput=False)
    # pre-transposed, mu1-folded x: xt16[p, t*64+m] = (x*mu1)[m, 128t+p]
    d_xt = nc.declare_dram_parameter("xt", [128, NT * M], F16, isOutput=False)
    d_ident = nc.declare_dram_parameter("ident", [128, 128], F16, isOutput=False)
    d_y = nc.declare_dram_parameter("y", [M, KL], F32, isOutput=True)

    mult = mybir.AluOpType.mult

    with TileContext(nc) as tc:
        with (
            tc.tile_pool(name="const", bufs=1) as constp,
            tc.tile_pool(name="stage", bufs=2) as stagep,
            tc.tile_pool(name="wpool", bufs=2) as wpool,
            tc.tile_pool(name="psum_t", bufs=2, space="PSUM") as psumt,
            tc.tile_pool(name="psum_y", bufs=2, space="PSUM") as psumy,
        ):
            ident = constp.tile([128, 128], F16)
            ones = constp.tile([1, M], F16)
            nc.vector.memset(ones[:], 1.0)

            s4d = constp.tile([128, NKT * NG * 2], F16)
            bias_sb = constp.tile([1, KL], F16)
            xT = constp.tile([128, NT * M], F16)
            # s4d rides the SWDGE queue ahead of the bulk W8 stream (the
            # first dequant needs it); everything else goes on sync.
            nc.gpsimd.dma_start(out=s4d[:], in_=d_s4d[:])
            nc.sync.dma_start(out=ident[:], in_=d_ident[:])
            nc.sync.dma_start(out=bias_sb[:], in_=d_bias[:])
            nc.sync.dma_start(out=xT[:], in_=d_xt[:])
            s4d_v = s4d.rearrange("p (g t) -> p g t", t=2)

            GW = 4            # t-columns gathered per transpose/evac tile
            evac_ctr = [0]
            b_off = [0]       # running column offset into the d_b stream

            for ks in range(2):             # k-super: 512 out cols of y
                y_ps = psumy.tile([M, 512], F32, tag="yps")
                nc.tensor.matmul(
                    y_ps[:], lhsT=ones[:],
                    rhs=bias_sb[:, ks * 512:(ks + 1) * 512],
                    start=True, stop=False,
                )
                mm = 0
                # software-pipeline: y-matmuls trail the transposes by one
                # gather tile so evacuation is off the PE critical path
                # (PE executes strictly in program order).
                pending = []

                def flush_mm(limit):
                    nonlocal mm
                    while len(pending) > limit:
                        wT, t0 = pending.pop(0)
                        for tl in range(GW):
                            mm += 1
                            nc.tensor.matmul(
                                y_ps[:],
                                lhsT=xT[:, (t0 + tl) * M:(t0 + tl + 1) * M],
                                rhs=wT[:, tl * 512:(tl + 1) * 512],
                                start=False, stop=(mm == NT),
                            )

                widths = WIDTHS0 if ks == 0 else WIDTHS1
                phases = []
                n0 = 0
                for nw_ in widths:
                    phases.append((n0, nw_))
                    n0 += nw_
                for n0, nw in phases:       # n phases
                    GH = nw // GS
                    # ONE cast-DMA (int8 -> f16, SWDGE queue) per phase
                    # covering all four k4 slabs — the host stream is laid
                    # out [ks][phase][k4-contiguous]
                    b_st = stagep.tile([128, 4 * nw], F16, tag="b", bufs=2)
                    nc.gpsimd.dma_start(
                        out=b_st[:], in_=d_b[:, b_off[0]:b_off[0] + 4 * nw])
                    b_off[0] += 4 * nw
                    b_stv = b_st.rearrange(
                        "p (k g r t) -> p k g r t", k=4, r=32, t=2)
                    w4 = []
                    for k4 in range(4):
                        kt = ks * 4 + k4
                        w = wpool.tile([128, nw], F16, tag=f"w{k4}", bufs=2)
                        goff = kt * NG + n0 // GS
                        sb = s4d_v[:, goff:goff + GH, :].unsqueeze(2).to_broadcast(
                            [128, GH, 32, 2])
                        w4v = w.rearrange("p (g r t) -> p g r t", r=32, t=2)
                        # the entire dequant: w = W8 * s4[k, g]
                        nc.vector.tensor_tensor(
                            out=w4v[:], in0=b_stv[:, k4], in1=sb[:], op=mult)
                        w4.append(w)
                    TH = nw // 128
                    for tg in range(TH // GW):
                        ps_t = psumt.tile([128, GW * 512], F16, tag="pst")
                        for tl in range(GW):
                            t = tg * GW + tl
                            for k4 in range(4):
                                nc.tensor.transpose(
                                    ps_t[:, (tl * 4 + k4) * 128:(tl * 4 + k4 + 1) * 128],
                                    w4[k4][:, t * 128:(t + 1) * 128],
                                    ident[:],
                                )
                        wT = stagep.tile([128, GW * 512], F16, tag="wT", bufs=3)
                        # evac: ScalarE mostly, every 3rd on the (light) DVE
                        if evac_ctr[0] % 3 == 2:
                            nc.vector.tensor_copy(wT[:], ps_t[:])
                        else:
                            nc.scalar.copy(wT[:], ps_t[:])
                        evac_ctr[0] += 1
                        pending.append((wT, n0 // 128 + tg * GW))
                        flush_mm(1)
                flush_mm(0)
                y_sb = stagep.tile([M, 512], F32, tag="ysb")
                nc.scalar.copy(y_sb[:], y_ps[:])
                nc.sync.dma_start(out=d_y[:, ks * 512:(ks + 1) * 512], in_=y_sb[:])

    nc.compile()
    return nc


def _get_compiled():
    global _compiled
    if _compiled is None:
        _compiled = _build()
    return _compiled


def make_in_maps(x, W_q, scales, zeros, mask, mu1, mu2, bias):
    x = np.asarray(x, dtype=np.float32)
    W_q = np.asarray(W_q, dtype=np.float32).reshape(K, N)
    scales = np.asarray(scales, dtype=np.float32).reshape(K, NG)
    zeros = np.asarray(zeros, dtype=np.float32).reshape(K, NG)
    mask_f = np.asarray(mask, dtype=np.float32)
    mu1 = np.asarray(mu1, dtype=np.float32)
    mu2 = np.asarray(mu2, dtype=np.float32)
    bias = np.asarray(bias, dtype=np.float32)

    # symmetric per-group re-encode: W8 = round(127 (q - z)/alpha) * mask,
    # alpha = max|q - z| over the group's kept weights
    qz = (W_q - np.repeat(zeros, GS, axis=1)) * mask_f        # [K, N]
    amax = np.abs(qz).reshape(K, NG, GS).max(axis=2)          # [K, NG]
    amax[amax == 0.0] = 1.0
    W8 = np.rint(qz * np.repeat(127.0 / amax, GS, axis=1)).astype(np.int8)
    sc4 = scales * (amax / 127.0) * mu2[:, None]              # folded scales

    # pre-transposed, mu1-folded x as f16
    xtp = np.ascontiguousarray(
        (x * mu1[None, :]).astype(np.float16).reshape(M, NT, 128)
        .transpose(2, 1, 0)).reshape(128, NT * M)

    # stream-order the W8 slabs: for (ks, phase, k4): [128, nw] with
    # partition p = k-row kt*128+p, columns n0:n0+nw
    def pack_b(b_r):
        bt = b_r.reshape(NKT, 128, N)  # [kt, p, n]
        slabs = []
        for ks, widths in ((0, WIDTHS0), (1, WIDTHS1)):
            n0 = 0
            for nw in widths:
                for k4 in range(4):
                    slabs.append(bt[ks * 4 + k4, :, n0:n0 + nw])
                n0 += nw
        return np.ascontiguousarray(np.concatenate(slabs, axis=1))

    in_maps = []
    for c in range(N_CORES):
        r = slice(c * KL, (c + 1) * KL)
        # s4d[p, (kt, g, t)] = sc4[kt*128+p, g] pair-duplicated along t
        sc_t = sc4[r].reshape(NKT, 128, NG).transpose(1, 0, 2)   # [128, NKT, NG]
        s4d = np.repeat(sc_t.reshape(128, NKT * NG), 2, axis=1).astype(np.float16)
        in_maps.append({
            "b": pack_b(W8[r]),
            "s4d": np.ascontiguousarray(s4d),
            "bias": np.ascontiguousarray(bias[r].reshape(1, KL).astype(np.float16)),
            "xt": xtp,
            "ident": np.eye(128, dtype=np.float16),
        })
    return in_maps


def kernel(x, W_q, scales, zeros, mask, mu1, mu2, bias, **run_kwargs):
    nc = _get_compiled()
    in_maps = make_in_maps(x, W_q, scales, zeros, mask, mu1, mu2, bias)
    res = bass_utils.run_bass_kernel_spmd(
        nc, in_maps, core_ids=list(range(N_CORES)), **run_kwargs
    )
    y = np.concatenate([res.results[c]["y"] for c in range(N_CORES)], axis=1)
    if run_kwargs:
        return y, res
    return y


# revision 12
# speedup vs baseline: 1.0823x; 1.0823x over previous
"""BCP quantized linear SPMD kernel for 8 Trainium2 NeuronCores.

Computes y = x @ W_deq.T + bias where
  W_deq = ((W_q - zeros) * scales) * mu2[:,None] * mu1[None,:] * mask

Sharding: tensor-parallel along the output dim K (8192 -> 1024 rows/core).
x and mu1 are replicated; the [64, 1024] per-core outputs are concatenated
on the host.

v5 dataflow: the host re-encodes the int4+zero-point+mask weights as
symmetric per-group int8:
    W8[k,n] = round(127 * (q - z) / alpha) * mask,
    alpha[k,g] = max|q - z| over the group (masked),
with alpha/127, mu2 and the quant scales folded into one per-group f16
scale tensor (uploaded pre-pair-duplicated for broadcast APs), and mu1
folded into a pre-transposed f16 x upload.  Under a symmetric encoding
the pruned weights are exactly 0, so the mask and the zero-point
subtraction vanish from the device inner loop:

  - one int8 stream cast-DMA'd to f16 SBUF tiles (k on partitions),
  - dequant = ONE pair-broadcast tensor_tensor (w = W8 * s4) per tile,
  - PE transposes [128,128] blocks in is_transpose mode (f16 PSUM out),
    16 blocks per [128, 2048] PSUM gather tile; evacuation alternates
    ScalarE / DVE; y[64, k] += xT.T @ wT accumulates per 512-wide
    k-superblock with the bias preloaded via a ones x bias matmul.
"""
import numpy as np

import concourse.bacc as bacc
import concourse.mybir as mybir
from concourse.tile import TileContext
from concourse import bass_utils

M = 64        # tokens
N = 8192      # in features
K = 8192      # out features
GS = 64       # quant group size
NG = N // GS  # 128 groups
N_CORES = 8
KL = K // N_CORES   # 1024 out rows per core
NKT = KL // 128     # 8 k tiles per core
NT = N // 128       # 64 n tiles
F16 = mybir.dt.float16
F32 = mybir.dt.float32
I8 = mybir.dt.int8

# n-phase widths per k-superblock (ks): small phases at the kernel's head
# and tail shrink pipeline fill/drain; large in the middle for DMA/DVE
# instruction efficiency.
WIDTHS0 = [512, 512, 1024, 2048, 4096]
WIDTHS1 = [4096, 2048, 1024, 512, 512]

_compiled = None


def _build():
    nc = bacc.Bacc("TRN2", target_bir_lowering=False)

    # W8 stream is pre-tiled on the host: one contiguous [128, nw] slab
    # per (ks, phase, k4) in emission order.
    d_b = nc.declare_dram_parameter("b", [128, NKT * N], I8, isOutput=False)
    # pair-duplicated per-(k,group) scales, host-folded:
    # s4d[p, kt*2NG + 2g + t] = scales*mu2*alpha/127
    d_s4d = nc.declare_dram_parameter("s4d", [128, NKT * NG * 2], F16, isOutput=False)
    d_bias = nc.declare_dram_parameter("bias", [1, KL], F16, isOutput=False)
    # pre-transposed, mu1-folded x: xt16[p, t*64+m] = (x*mu1)[m, 128t+p]
    d_xt = nc.declare_dram_parameter("xt", [128, NT * M], F16, isOutput=False)
    d_ident = nc.declare_dram_parameter("ident", [128, 128], F16, isOutput=False)
    d_y = nc.declare_dram_parameter("y", [M, KL], F32, isOutput=True)

    mult = mybir.AluOpType.mult

    with TileContext(nc) as tc:
        with (
            tc.tile_pool(name="const", bufs=1) as constp,
            tc.tile_pool(name="stage", bufs=2) as stagep,
            tc.tile_pool(name="wpool", bufs=2) as wpool,
            tc.tile_pool(name="psum_t", bufs=2, space="PSUM") as psumt,
            tc.tile_pool(name="psum_y", bufs=2, space="PSUM") as psumy,
        ):
            ident = constp.tile([128, 128], F16)
            ones = constp.tile([1, M], F16)
            nc.vector.memset(ones[:], 1.0)

            s4d = constp.tile([128, NKT * NG * 2], F16)
            bias_sb = constp.tile([1, KL], F16)
            xT = constp.tile([128, NT * M], F16)
            # s4d rides the SWDGE queue ahead of the bulk W8 stream (the
            # first dequant needs it); everything else goes on sync.
            nc.gpsimd.dma_start(out=s4d[:], in_=d_s4d[:])
            nc.sync.dma_start(out=ident[:], in_=d_ident[:])
            nc.sync.dma_start(out=bias_sb[:], in_=d_bias[:])
            nc.sync.dma_start(out=xT[:], in_=d_xt[:])
            s4d_v = s4d.rearrange("p (g t) -> p g t", t=2)

            GW = 4            # t-columns gathered per transpose/evac tile
            evac_ctr = [0]
            b_off = [0]       # running column offset into the d_b stream

            for ks in range(2):             # k-super: 512 out cols of y
                y_ps = psumy.tile([M, 512], F32, tag="yps")
                nc.tensor.matmul(
                    y_ps[:], lhsT=ones[:],
                    rhs=bias_sb[:, ks * 512:(ks + 1) * 512],
                    start=True, stop=False,
                )
                mm = 0
                # software-pipeline: y-matmuls trail the transposes by one
                # gather tile so evacuation is off the PE critical path
                # (PE executes strictly in program order).
                pending = []

                def flush_mm(limit):
                    nonlocal mm
                    while len(pending) > limit:
                        wT, t0 = pending.pop(0)
                        for tl in range(GW):
                            mm += 1
                            nc.tensor.matmul(
                                y_ps[:],
                                lhsT=xT[:, (t0 + tl) * M:(t0 + tl + 1) * M],
                                rhs=wT[:, tl * 512:(tl + 1) * 512],
                                start=False, stop=(mm == NT),
                            )

                widths = WIDTHS0 if ks == 0 else WIDTHS1
                phases = []
                n0 = 0
                for nw_ in widths:
                    phases.append((n0, nw_))
                    n0 += nw_
                for n0, nw in phases:       # n phases
                    GH = nw // GS
                    w4 = []
                    for k4 in range(4):
                        kt = ks * 4 + k4
                        b_st = stagep.tile([128, nw], F16, tag=f"b{k4}", bufs=2)
                        src = d_b[:, b_off[0]:b_off[0] + nw]
                        b_off[0] += nw
                        # cast-DMAs (int8 -> f16) ride the SWDGE queue
                        nc.gpsimd.dma_start(out=b_st[:], in_=src)
                        w = wpool.tile([128, nw], F16, tag=f"w{k4}", bufs=2)
                        goff = kt * NG + n0 // GS
                        sb = s4d_v[:, goff:goff + GH, :].unsqueeze(2).to_broadcast(
                            [128, GH, 32, 2])
                        b4 = b_st.rearrange("p (g r t) -> p g r t", r=32, t=2)
                        w4v = w.rearrange("p (g r t) -> p g r t", r=32, t=2)
                        # the entire dequant: w = W8 * s4[k, g]
                        nc.vector.tensor_tensor(out=w4v[:], in0=b4[:], in1=sb[:], op=mult)
                        w4.append(w)
                    TH = nw // 128
                    for tg in range(TH // GW):
                        ps_t = psumt.tile([128, GW * 512], F16, tag="pst")
                        for tl in range(GW):
                            t = tg * GW + tl
                            for k4 in range(4):
                                nc.tensor.transpose(
                                    ps_t[:, (tl * 4 + k4) * 128:(tl * 4 + k4 + 1) * 128],
                                    w4[k4][:, t * 128:(t + 1) * 128],
                                    ident[:],
                                )
                        wT = stagep.tile([128, GW * 512], F16, tag="wT", bufs=3)
                        # evac: ScalarE mostly, every 3rd on the (light) DVE
                        if evac_ctr[0] % 3 == 2:
                            nc.vector.tensor_copy(wT[:], ps_t[:])
                        else:
                            nc.scalar.copy(wT[:], ps_t[:])
                        evac_ctr[0] += 1
                        pending.append((wT, n0 // 128 + tg * GW))
                        flush_mm(1)
                flush_mm(0)
                y_sb = stagep.tile([M, 512], F32, tag="ysb")
                nc.scalar.copy(y_sb[:], y_ps[:])
                nc.sync.dma_start(out=d_y[:, ks * 512:(ks + 1) * 512], in_=y_sb[:])

    nc.compile()
    return nc


def _get_compiled():
    global _compiled
    if _compiled is None:
        _compiled = _build()
    return _compiled


def make_in_maps(x, W_q, scales, zeros, mask, mu1, mu2, bias):
    x = np.asarray(x, dtype=np.float32)
    W_q = np.asarray(W_q, dtype=np.float32).reshape(K, N)
    scales = np.asarray(scales, dtype=np.float32).reshape(K, NG)
    zeros = np.asarray(zeros, dtype=np.float32).reshape(K, NG)
    mask_f = np.asarray(mask, dtype=np.float32)
    mu1 = np.asarray(mu1, dtype=np.float32)
    mu2 = np.asarray(mu2, dtype=np.float32)
    bias = np.asarray(bias, dtype=np.float32)

    # symmetric per-group re-encode: W8 = round(127 (q - z)/alpha) * mask,
    # alpha = max|q - z| over the group's kept weights
    qz = (W_q - np.repeat(zeros, GS, axis=1)) * mask_f        # [K, N]
    amax = np.abs(qz).reshape(K, NG, GS).max(axis=2)          # [K, NG]
    amax[amax == 0.0] = 1.0
    W8 = np.rint(qz * np.repeat(127.0 / amax, GS, axis=1)).astype(np.int8)
    sc4 = scales * (amax / 127.0) * mu2[:, None]              # folded scales

    # pre-transposed, mu1-folded x as f16
    xtp = np.ascontiguousarray(
        (x * mu1[None, :]).astype(np.float16).reshape(M, NT, 128)
        .transpose(2, 1, 0)).reshape(128, NT * M)

    # stream-order the W8 slabs: for (ks, phase, k4): [128, nw] with
    # partition p = k-row kt*128+p, columns n0:n0+nw
    def pack_b(b_r):
        bt = b_r.reshape(NKT, 128, N)  # [kt, p, n]
        slabs = []
        for ks, widths in ((0, WIDTHS0), (1, WIDTHS1)):
            n0 = 0
            for nw in widths:
                for k4 in range(4):
                    slabs.append(bt[ks * 4 + k4, :, n0:n0 + nw])
                n0 += nw
        return np.ascontiguousarray(np.concatenate(slabs, axis=1))

    in_maps = []
    for c in range(N_CORES):
        r = slice(c * KL, (c + 1) * KL)
        # s4d[p, (kt, g, t)] = sc4[kt*128+p, g] pair-duplicated along t
        sc_t = sc4[r].reshape(NKT, 128, NG).transpose(1, 0, 2)   # [128, NKT, NG]
        s4d = np.repeat(sc_t.reshape(128, NKT * NG), 2, axis=1).astype(np.float16)
        in_maps.append({
            "b": pack_b(W8[r]),
            "s4d": np.ascontiguousarray(s4d),
            "bias": np.ascontiguousarray(bias[r].reshape(1, KL).astype(np.float16)),
            "xt": xtp,
            "ident": np.eye(128, dtype=np.float16),
        })
    return in_maps


def kernel(x, W_q, scales, zeros, mask, mu1, mu2, bias, **run_kwargs):
    nc = _get_compiled()
    in_maps = make_in_maps(x, W_q, scales, zeros, mask, mu1, mu2, bias)
    res = bass_utils.run_bass_kernel_spmd(
        nc, in_maps, core_ids=list(range(N_CORES)), **run_kwargs
    )
    y = np.concatenate([res.results[c]["y"] for c in range(N_CORES)], axis=1)
    if run_kwargs:
        return y, res
    return y


# revision 13
# speedup vs baseline: 1.0850x; 1.0024x over previous
"""BCP quantized linear SPMD kernel for 8 Trainium2 NeuronCores.

Computes y = x @ W_deq.T + bias where
  W_deq = ((W_q - zeros) * scales) * mu2[:,None] * mu1[None,:] * mask

Sharding: tensor-parallel along the output dim K (8192 -> 1024 rows/core).
x and mu1 are replicated; the [64, 1024] per-core outputs are concatenated
on the host.

v5 dataflow: the host re-encodes the int4+zero-point+mask weights as
symmetric per-group int8:
    W8[k,n] = round(127 * (q - z) / alpha) * mask,
    alpha[k,g] = max|q - z| over the group (masked),
with alpha/127, mu2 and the quant scales folded into one per-group f16
scale tensor (uploaded pre-pair-duplicated for broadcast APs), and mu1
folded into a pre-transposed f16 x upload.  Under a symmetric encoding
the pruned weights are exactly 0, so the mask and the zero-point
subtraction vanish from the device inner loop:

  - one int8 stream cast-DMA'd to f16 SBUF tiles (k on partitions),
  - dequant = ONE pair-broadcast tensor_tensor (w = W8 * s4) per tile,
  - PE transposes [128,128] blocks in is_transpose mode (f16 PSUM out),
    16 blocks per [128, 2048] PSUM gather tile; evacuation alternates
    ScalarE / DVE; y[64, k] += xT.T @ wT accumulates per 512-wide
    k-superblock with the bias preloaded via a ones x bias matmul.
"""
import numpy as np

import concourse.bacc as bacc
import concourse.mybir as mybir
from concourse.tile import TileContext
from concourse import bass_utils

M = 64        # tokens
N = 8192      # in features
K = 8192      # out features
GS = 64       # quant group size
NG = N // GS  # 128 groups
N_CORES = 8
KL = K // N_CORES   # 1024 out rows per core
NKT = KL // 128     # 8 k tiles per core
NT = N // 128       # 64 n tiles
F16 = mybir.dt.float16
F32 = mybir.dt.float32
I8 = mybir.dt.int8

# n-phase widths per k-superblock (ks): small phases at the kernel's head
# and tail shrink pipeline fill/drain; large in the middle for DMA/DVE
# instruction efficiency.
WIDTHS0 = [512, 512, 1024, 2048, 4096]
WIDTHS1 = [4096, 2048, 1024, 512, 512]

_compiled = None


def _build():
    nc = bacc.Bacc("TRN2", target_bir_lowering=False)

    # W8 stream is pre-tiled on the host: one contiguous [128, nw] slab
    # per (ks, phase, k4) in emission order.
    d_b = nc.declare_dram_parameter("b", [128, NKT * N], I8, isOutput=False)
    # pair-duplicated per-(k,group) scales, host-folded:
    # s4d[p, kt*2NG + 2g + t] = scales*mu2*alpha/127
    d_s4d = nc.declare_dram_parameter("s4d", [128, NKT * NG * 2], F16, isOutput=False)
    d_bias = nc.declare_dram_parameter("bias", [1, KL], F16, isOutput=False)
    # pre-transposed, mu1-folded x: xt16[p, t*64+m] = (x*mu1)[m, 128t+p]
    d_xt = nc.declare_dram_parameter("xt", [128, NT * M], F16, isOutput=False)
    d_ident = nc.declare_dram_parameter("ident", [128, 128], F16, isOutput=False)
    d_y = nc.declare_dram_parameter("y", [M, KL], F32, isOutput=True)

    mult = mybir.AluOpType.mult

    with TileContext(nc) as tc:
        with (
            tc.tile_pool(name="const", bufs=1) as constp,
            tc.tile_pool(name="stage", bufs=2) as stagep,
            tc.tile_pool(name="wpool", bufs=2) as wpool,
            tc.tile_pool(name="psum_t", bufs=2, space="PSUM") as psumt,
            tc.tile_pool(name="psum_y", bufs=2, space="PSUM") as psumy,
        ):
            ident = constp.tile([128, 128], F16)
            ones = constp.tile([1, M], F16)
            nc.vector.memset(ones[:], 1.0)

            s4d = constp.tile([128, NKT * NG * 2], F16)
            bias_sb = constp.tile([1, KL], F16)
            xT = constp.tile([128, NT * M], F16)
            # non-cast DMAs ride the sync queue, keeping the SWDGE queue
            # free for the W8 stream from instruction 0
            nc.sync.dma_start(out=s4d[:], in_=d_s4d[:])
            nc.sync.dma_start(out=ident[:], in_=d_ident[:])
            nc.sync.dma_start(out=bias_sb[:], in_=d_bias[:])
            nc.sync.dma_start(out=xT[:], in_=d_xt[:])
            s4d_v = s4d.rearrange("p (g t) -> p g t", t=2)

            GW = 4            # t-columns gathered per transpose/evac tile
            evac_ctr = [0]
            b_off = [0]       # running column offset into the d_b stream

            for ks in range(2):             # k-super: 512 out cols of y
                y_ps = psumy.tile([M, 512], F32, tag="yps")
                nc.tensor.matmul(
                    y_ps[:], lhsT=ones[:],
                    rhs=bias_sb[:, ks * 512:(ks + 1) * 512],
                    start=True, stop=False,
                )
                mm = 0
                # software-pipeline: y-matmuls trail the transposes by one
                # gather tile so evacuation is off the PE critical path
                # (PE executes strictly in program order).
                pending = []

                def flush_mm(limit):
                    nonlocal mm
                    while len(pending) > limit:
                        wT, t0 = pending.pop(0)
                        for tl in range(GW):
                            mm += 1
                            nc.tensor.matmul(
                                y_ps[:],
                                lhsT=xT[:, (t0 + tl) * M:(t0 + tl + 1) * M],
                                rhs=wT[:, tl * 512:(tl + 1) * 512],
                                start=False, stop=(mm == NT),
                            )

                widths = WIDTHS0 if ks == 0 else WIDTHS1
                phases = []
                n0 = 0
                for nw_ in widths:
                    phases.append((n0, nw_))
                    n0 += nw_
                for n0, nw in phases:       # n phases
                    GH = nw // GS
                    w4 = []
                    for k4 in range(4):
                        kt = ks * 4 + k4
                        b_st = stagep.tile([128, nw], F16, tag=f"b{k4}", bufs=2)
                        src = d_b[:, b_off[0]:b_off[0] + nw]
                        b_off[0] += nw
                        # cast-DMAs (int8 -> f16) ride the SWDGE queue
                        nc.gpsimd.dma_start(out=b_st[:], in_=src)
                        w = wpool.tile([128, nw], F16, tag=f"w{k4}", bufs=3)
                        goff = kt * NG + n0 // GS
                        sb = s4d_v[:, goff:goff + GH, :].unsqueeze(2).to_broadcast(
                            [128, GH, 32, 2])
                        b4 = b_st.rearrange("p (g r t) -> p g r t", r=32, t=2)
                        w4v = w.rearrange("p (g r t) -> p g r t", r=32, t=2)
                        # the entire dequant: w = W8 * s4[k, g]
                        nc.vector.tensor_tensor(out=w4v[:], in0=b4[:], in1=sb[:], op=mult)
                        w4.append(w)
                    TH = nw // 128
                    for tg in range(TH // GW):
                        ps_t = psumt.tile([128, GW * 512], F16, tag="pst")
                        for tl in range(GW):
                            t = tg * GW + tl
                            for k4 in range(4):
                                nc.tensor.transpose(
                                    ps_t[:, (tl * 4 + k4) * 128:(tl * 4 + k4 + 1) * 128],
                                    w4[k4][:, t * 128:(t + 1) * 128],
                                    ident[:],
                                )
                        wT = stagep.tile([128, GW * 512], F16, tag="wT", bufs=3)
                        # all evacs on ScalarE: DVE stays a pure dequant
                        # stream so next-phase prefetch is never blocked
                        nc.scalar.copy(wT[:], ps_t[:])
                        evac_ctr[0] += 1
                        pending.append((wT, n0 // 128 + tg * GW))
                        flush_mm(1)
                flush_mm(0)
                y_sb = stagep.tile([M, 512], F32, tag="ysb")
                nc.scalar.copy(y_sb[:], y_ps[:])
                nc.sync.dma_start(out=d_y[:, ks * 512:(ks + 1) * 512], in_=y_sb[:])

    nc.compile()
    return nc


def _get_compiled():
    global _compiled
    if _compiled is None:
        _compiled = _build()
    return _compiled


def make_in_maps(x, W_q, scales, zeros, mask, mu1, mu2, bias):
    x = np.asarray(x, dtype=np.float32)
    W_q = np.asarray(W_q, dtype=np.float32).reshape(K, N)
    scales = np.asarray(scales, dtype=np.float32).reshape(K, NG)
    zeros = np.asarray(zeros, dtype=np.float32).reshape(K, NG)
    mask_f = np.asarray(mask, dtype=np.float32)
    mu1 = np.asarray(mu1, dtype=np.float32)
    mu2 = np.asarray(mu2, dtype=np.float32)
    bias = np.asarray(bias, dtype=np.float32)

    # symmetric per-group re-encode: W8 = round(127 (q - z)/alpha) * mask,
    # alpha = max|q - z| over the group's kept weights
    qz = (W_q - np.repeat(zeros, GS, axis=1)) * mask_f        # [K, N]
    amax = np.abs(qz).reshape(K, NG, GS).max(axis=2)          # [K, NG]
    amax[amax == 0.0] = 1.0
    W8 = np.rint(qz * np.repeat(127.0 / amax, GS, axis=1)).astype(np.int8)
    sc4 = scales * (amax / 127.0) * mu2[:, None]              # folded scales

    # pre-transposed, mu1-folded x as f16
    xtp = np.ascontiguousarray(
        (x * mu1[None, :]).astype(np.float16).reshape(M, NT, 128)
        .transpose(2, 1, 0)).reshape(128, NT * M)

    # stream-order the W8 slabs: for (ks, phase, k4): [128, nw] with
    # partition p = k-row kt*128+p, columns n0:n0+nw
    def pack_b(b_r):
        bt = b_r.reshape(NKT, 128, N)  # [kt, p, n]
        slabs = []
        for ks, widths in ((0, WIDTHS0), (1, WIDTHS1)):
            n0 = 0
            for nw in widths:
                for k4 in range(4):
                    slabs.append(bt[ks * 4 + k4, :, n0:n0 + nw])
                n0 += nw
        return np.ascontiguousarray(np.concatenate(slabs, axis=1))

    in_maps = []
    for c in range(N_CORES):
        r = slice(c * KL, (c + 1) * KL)
        # s4d[p, (kt, g, t)] = sc4[kt*128+p, g] pair-duplicated along t
        sc_t = sc4[r].reshape(NKT, 128, NG).transpose(1, 0, 2)   # [128, NKT, NG]
        s4d = np.repeat(sc_t.reshape(128, NKT * NG), 2, axis=1).astype(np.float16)
        in_maps.append({
            "b": pack_b(W8[r]),
            "s4d": np.ascontiguousarray(s4d),
            "bias": np.ascontiguousarray(bias[r].reshape(1, KL).astype(np.float16)),
            "xt": xtp,
            "ident": np.eye(128, dtype=np.float16),
        })
    return in_maps


def kernel(x, W_q, scales, zeros, mask, mu1, mu2, bias, **run_kwargs):
    nc = _get_compiled()
    in_maps = make_in_maps(x, W_q, scales, zeros, mask, mu1, mu2, bias)
    res = bass_utils.run_bass_kernel_spmd(
        nc, in_maps, core_ids=list(range(N_CORES)), **run_kwargs
    )
    y = np.concatenate([res.results[c]["y"] for c in range(N_CORES)], axis=1)
    if run_kwargs:
        return y, res
    return y


# revision 14
# speedup vs baseline: 1.0981x; 1.0121x over previous
"""BCP quantized linear SPMD kernel for 8 Trainium2 NeuronCores.

Computes y = x @ W_deq.T + bias where
  W_deq = ((W_q - zeros) * scales) * mu2[:,None] * mu1[None,:] * mask

Sharding: tensor-parallel along the output dim K (8192 -> 1024 rows/core).
x and mu1 are replicated; the [64, 1024] per-core outputs are concatenated
on the host.

v5 dataflow: the host re-encodes the int4+zero-point+mask weights as
symmetric per-group int8:
    W8[k,n] = round(127 * (q - z) / alpha) * mask,
    alpha[k,g] = max|q - z| over the group (masked),
with alpha/127, mu2 and the quant scales folded into one per-group f16
scale tensor (uploaded pre-pair-duplicated for broadcast APs), and mu1
folded into a pre-transposed f16 x upload.  Under a symmetric encoding
the pruned weights are exactly 0, so the mask and the zero-point
subtraction vanish from the device inner loop:

  - one int8 stream cast-DMA'd to f16 SBUF tiles (k on partitions),
  - dequant = ONE pair-broadcast tensor_tensor (w = W8 * s4) per tile,
  - PE transposes [128,128] blocks in is_transpose mode (f16 PSUM out),
    16 blocks per [128, 2048] PSUM gather tile; evacuation alternates
    ScalarE / DVE; y[64, k] += xT.T @ wT accumulates per 512-wide
    k-superblock with the bias preloaded via a ones x bias matmul.
"""
import numpy as np

import concourse.bacc as bacc
import concourse.mybir as mybir
from concourse.tile import TileContext
from concourse import bass_utils

M = 64        # tokens
N = 8192      # in features
K = 8192      # out features
GS = 64       # quant group size
NG = N // GS  # 128 groups
N_CORES = 8
KL = K // N_CORES   # 1024 out rows per core
NKT = KL // 128     # 8 k tiles per core
NT = N // 128       # 64 n tiles
F16 = mybir.dt.float16
F32 = mybir.dt.float32
I8 = mybir.dt.int8

# n-phase widths per k-superblock (ks): small phases at the kernel's head
# and tail shrink pipeline fill/drain; large in the middle for DMA/DVE
# instruction efficiency.
WIDTHS0 = [512, 512, 1024, 2048, 4096]
WIDTHS1 = [4096, 2048, 1024, 512, 512]

_compiled = None


def _build():
    nc = bacc.Bacc("TRN2", target_bir_lowering=False)

    # W8 stream is pre-tiled on the host: one contiguous [128, nw] slab
    # per (ks, phase, k4) in emission order.
    d_b = nc.declare_dram_parameter("b", [128, NKT * N], I8, isOutput=False)
    # pair-duplicated per-(k,group) scales, host-folded:
    # s4d[p, kt*2NG + 2g + t] = scales*mu2*alpha/127
    d_s4d = nc.declare_dram_parameter("s4d", [128, NKT * NG * 2], F16, isOutput=False)
    d_bias = nc.declare_dram_parameter("bias", [1, KL], F16, isOutput=False)
    # pre-transposed, mu1-folded x: xt16[p, t*64+m] = (x*mu1)[m, 128t+p]
    d_xt = nc.declare_dram_parameter("xt", [128, NT * M], F16, isOutput=False)
    d_ident = nc.declare_dram_parameter("ident", [128, 128], F16, isOutput=False)
    d_y = nc.declare_dram_parameter("y", [M, KL], F32, isOutput=True)

    mult = mybir.AluOpType.mult

    with TileContext(nc) as tc:
        with (
            tc.tile_pool(name="const", bufs=1) as constp,
            tc.tile_pool(name="stage", bufs=2) as stagep,
            tc.tile_pool(name="wpool", bufs=2) as wpool,
            tc.tile_pool(name="psum_t", bufs=2, space="PSUM") as psumt,
            tc.tile_pool(name="psum_y", bufs=2, space="PSUM") as psumy,
        ):
            ident = constp.tile([128, 128], F16)
            ones = constp.tile([1, M], F16)
            nc.vector.memset(ones[:], 1.0)

            s4d = constp.tile([128, NKT * NG * 2], F16)
            bias_sb = constp.tile([1, KL], F16)
            xT = constp.tile([128, NT * M], F16)
            # non-cast DMAs ride the sync queue, keeping the SWDGE queue
            # free for the W8 stream from instruction 0.  The 1MB xT load is
            # deferred until after phase 0's slabs so it doesn't hog the DMA
            # engines during pipeline fill (xT is first read ~8us in).
            nc.sync.dma_start(out=s4d[:], in_=d_s4d[:])
            nc.sync.dma_start(out=bias_sb[:], in_=d_bias[:])
            nc.sync.dma_start(out=ident[:], in_=d_ident[:])
            s4d_v = s4d.rearrange("p (g t) -> p g t", t=2)

            GW = 4            # t-columns gathered per transpose/evac tile
            evac_ctr = [0]
            b_off = [0]       # running column offset into the d_b stream

            for ks in range(2):             # k-super: 512 out cols of y
                y_ps = psumy.tile([M, 512], F32, tag="yps")
                nc.tensor.matmul(
                    y_ps[:], lhsT=ones[:],
                    rhs=bias_sb[:, ks * 512:(ks + 1) * 512],
                    start=True, stop=False,
                )
                mm = 0
                # software-pipeline: y-matmuls trail the transposes by one
                # gather tile so evacuation is off the PE critical path
                # (PE executes strictly in program order).
                pending = []

                def flush_mm(limit):
                    nonlocal mm
                    while len(pending) > limit:
                        wT, t0 = pending.pop(0)
                        for tl in range(GW):
                            mm += 1
                            nc.tensor.matmul(
                                y_ps[:],
                                lhsT=xT[:, (t0 + tl) * M:(t0 + tl + 1) * M],
                                rhs=wT[:, tl * 512:(tl + 1) * 512],
                                start=False, stop=(mm == NT),
                            )

                widths = WIDTHS0 if ks == 0 else WIDTHS1
                phases = []
                n0 = 0
                for nw_ in widths:
                    phases.append((n0, nw_))
                    n0 += nw_
                for n0, nw in phases:       # n phases
                    GH = nw // GS
                    w4 = []
                    for k4 in range(4):
                        kt = ks * 4 + k4
                        b_st = stagep.tile([128, nw], F16, tag=f"b{k4}", bufs=2)
                        src = d_b[:, b_off[0]:b_off[0] + nw]
                        b_off[0] += nw
                        # cast-DMAs (int8 -> f16) ride the SWDGE queue
                        nc.gpsimd.dma_start(out=b_st[:], in_=src)
                        w = wpool.tile([128, nw], F16, tag=f"w{k4}", bufs=3)
                        goff = kt * NG + n0 // GS
                        sb = s4d_v[:, goff:goff + GH, :].unsqueeze(2).to_broadcast(
                            [128, GH, 32, 2])
                        b4 = b_st.rearrange("p (g r t) -> p g r t", r=32, t=2)
                        w4v = w.rearrange("p (g r t) -> p g r t", r=32, t=2)
                        # the entire dequant: w = W8 * s4[k, g]
                        nc.vector.tensor_tensor(out=w4v[:], in0=b4[:], in1=sb[:], op=mult)
                        w4.append(w)
                    if ks == 0 and n0 == 0:
                        nc.sync.dma_start(out=xT[:], in_=d_xt[:])
                    TH = nw // 128
                    for tg in range(TH // GW):
                        ps_t = psumt.tile([128, GW * 512], F16, tag="pst")
                        # k4-major: the first transposes of a phase only need
                        # the k4=0 dequant, smoothing pipeline fill
                        for k4 in range(4):
                            for tl in range(GW):
                                t = tg * GW + tl
                                nc.tensor.transpose(
                                    ps_t[:, (tl * 4 + k4) * 128:(tl * 4 + k4 + 1) * 128],
                                    w4[k4][:, t * 128:(t + 1) * 128],
                                    ident[:],
                                )
                        wT = stagep.tile([128, GW * 512], F16, tag="wT", bufs=3)
                        # all evacs on ScalarE: DVE stays a pure dequant
                        # stream so next-phase prefetch is never blocked
                        nc.scalar.copy(wT[:], ps_t[:])
                        evac_ctr[0] += 1
                        pending.append((wT, n0 // 128 + tg * GW))
                        flush_mm(1)
                flush_mm(0)
                y_sb = stagep.tile([M, 512], F32, tag="ysb")
                nc.scalar.copy(y_sb[:], y_ps[:])
                nc.sync.dma_start(out=d_y[:, ks * 512:(ks + 1) * 512], in_=y_sb[:])

    nc.compile()
    return nc


def _get_compiled():
    global _compiled
    if _compiled is None:
        _compiled = _build()
    return _compiled


def make_in_maps(x, W_q, scales, zeros, mask, mu1, mu2, bias):
    x = np.asarray(x, dtype=np.float32)
    W_q = np.asarray(W_q, dtype=np.float32).reshape(K, N)
    scales = np.asarray(scales, dtype=np.float32).reshape(K, NG)
    zeros = np.asarray(zeros, dtype=np.float32).reshape(K, NG)
    mask_f = np.asarray(mask, dtype=np.float32)
    mu1 = np.asarray(mu1, dtype=np.float32)
    mu2 = np.asarray(mu2, dtype=np.float32)
    bias = np.asarray(bias, dtype=np.float32)

    # symmetric per-group re-encode: W8 = round(127 (q - z)/alpha) * mask,
    # alpha = max|q - z| over the group's kept weights
    qz = (W_q - np.repeat(zeros, GS, axis=1)) * mask_f        # [K, N]
    amax = np.abs(qz).reshape(K, NG, GS).max(axis=2)          # [K, NG]
    amax[amax == 0.0] = 1.0
    W8 = np.rint(qz * np.repeat(127.0 / amax, GS, axis=1)).astype(np.int8)
    sc4 = scales * (amax / 127.0) * mu2[:, None]              # folded scales

    # pre-transposed, mu1-folded x as f16
    xtp = np.ascontiguousarray(
        (x * mu1[None, :]).astype(np.float16).reshape(M, NT, 128)
        .transpose(2, 1, 0)).reshape(128, NT * M)

    # stream-order the W8 slabs: for (ks, phase, k4): [128, nw] with
    # partition p = k-row kt*128+p, columns n0:n0+nw
    def pack_b(b_r):
        bt = b_r.reshape(NKT, 128, N)  # [kt, p, n]
        slabs = []
        for ks, widths in ((0, WIDTHS0), (1, WIDTHS1)):
            n0 = 0
            for nw in widths:
                for k4 in range(4):
                    slabs.append(bt[ks * 4 + k4, :, n0:n0 + nw])
                n0 += nw
        return np.ascontiguousarray(np.concatenate(slabs, axis=1))

    in_maps = []
    for c in range(N_CORES):
        r = slice(c * KL, (c + 1) * KL)
        # s4d[p, (kt, g, t)] = sc4[kt*128+p, g] pair-duplicated along t
        sc_t = sc4[r].reshape(NKT, 128, NG).transpose(1, 0, 2)   # [128, NKT, NG]
        s4d = np.repeat(sc_t.reshape(128, NKT * NG), 2, axis=1).astype(np.float16)
        in_maps.append({
            "b": pack_b(W8[r]),
            "s4d": np.ascontiguousarray(s4d),
            "bias": np.ascontiguousarray(bias[r].reshape(1, KL).astype(np.float16)),
            "xt": xtp,
            "ident": np.eye(128, dtype=np.float16),
        })
    return in_maps


def kernel(x, W_q, scales, zeros, mask, mu1, mu2, bias, **run_kwargs):
    nc = _get_compiled()
    in_maps = make_in_maps(x, W_q, scales, zeros, mask, mu1, mu2, bias)
    res = bass_utils.run_bass_kernel_spmd(
        nc, in_maps, core_ids=list(range(N_CORES)), **run_kwargs
    )
    y = np.concatenate([res.results[c]["y"] for c in range(N_CORES)], axis=1)
    if run_kwargs:
        return y, res
    return y


# revision 15
# speedup vs baseline: 1.1429x; 1.0408x over previous
"""BCP quantized linear SPMD kernel for 8 Trainium2 NeuronCores.

Computes y = x @ W_deq.T + bias where
  W_deq = ((W_q - zeros) * scales) * mu2[:,None] * mu1[None,:] * mask

Sharding: tensor-parallel along the output dim K (8192 -> 1024 rows/core).
x and mu1 are replicated; the [64, 1024] per-core outputs are concatenated
on the host.

v5 dataflow: the host re-encodes the int4+zero-point+mask weights as
symmetric per-group int8:
    W8[k,n] = round(127 * (q - z) / alpha) * mask,
    alpha[k,g] = max|q - z| over the group (masked),
with alpha/127, mu2 and the quant scales folded into one per-group f16
scale tensor (uploaded pre-pair-duplicated for broadcast APs), and mu1
folded into a pre-transposed f16 x upload.  Under a symmetric encoding
the pruned weights are exactly 0, so the mask and the zero-point
subtraction vanish from the device inner loop:

  - one int8 stream cast-DMA'd to f16 SBUF tiles (k on partitions),
  - dequant = ONE pair-broadcast tensor_tensor (w = W8 * s4) per tile,
  - PE transposes [128,128] blocks in is_transpose mode (f16 PSUM out),
    16 blocks per [128, 2048] PSUM gather tile; evacuation alternates
    ScalarE / DVE; y[64, k] += xT.T @ wT accumulates per 512-wide
    k-superblock with the bias preloaded via a ones x bias matmul.
"""
import numpy as np

import concourse.bacc as bacc
import concourse.mybir as mybir
from concourse.tile import TileContext
from concourse import bass_utils

M = 64        # tokens
N = 8192      # in features
K = 8192      # out features
GS = 64       # quant group size
NG = N // GS  # 128 groups
N_CORES = 8
KL = K // N_CORES   # 1024 out rows per core
NKT = KL // 128     # 8 k tiles per core
NT = N // 128       # 64 n tiles
F16 = mybir.dt.float16
F32 = mybir.dt.float32
I8 = mybir.dt.int8

# n-phase widths per k-superblock (ks): small phases at the kernel's head
# and tail shrink pipeline fill/drain; large in the middle for DMA/DVE
# instruction efficiency.
WIDTHS0 = [512, 512, 1024, 2048, 4096]
WIDTHS1 = [4096, 2048, 1024, 512, 512]

_compiled = None


def _build():
    nc = bacc.Bacc("TRN2", target_bir_lowering=False)

    # W8 stream is pre-tiled on the host: one contiguous [128, nw] slab
    # per (ks, phase, k4) in emission order.
    d_b = nc.declare_dram_parameter("b", [128, NKT * N], I8, isOutput=False)
    # pair-duplicated per-(k,group) scales, host-folded:
    # s4d[p, kt*2NG + 2g + t] = scales*mu2*alpha/127
    d_s4d = nc.declare_dram_parameter("s4d", [128, NKT * NG * 2], F16, isOutput=False)
    d_bias = nc.declare_dram_parameter("bias", [1, KL], F16, isOutput=False)
    # pre-transposed, mu1-folded x: xt16[p, t*64+m] = (x*mu1)[m, 128t+p]
    d_xt = nc.declare_dram_parameter("xt", [128, NT * M], F16, isOutput=False)
    d_ident = nc.declare_dram_parameter("ident", [128, 128], F16, isOutput=False)
    d_y = nc.declare_dram_parameter("y", [M, KL], F32, isOutput=True)

    mult = mybir.AluOpType.mult

    with TileContext(nc) as tc:
        with (
            tc.tile_pool(name="const", bufs=1) as constp,
            tc.tile_pool(name="stage", bufs=2) as stagep,
            tc.tile_pool(name="wpool", bufs=2) as wpool,
            tc.tile_pool(name="psum_t", bufs=2, space="PSUM") as psumt,
            tc.tile_pool(name="psum_y", bufs=2, space="PSUM") as psumy,
        ):
            ident = constp.tile([128, 128], F16)
            ones = constp.tile([1, M], F16)
            nc.vector.memset(ones[:], 1.0)

            s4d = constp.tile([128, NKT * NG * 2], F16)
            bias_sb = constp.tile([1, KL], F16)
            xT = constp.tile([128, NT * M], F16)
            # non-cast DMAs ride the sync queue, keeping the SWDGE queue
            # free for the W8 stream from instruction 0.  The 1MB xT load is
            # deferred until after phase 0's slabs so it doesn't hog the DMA
            # engines during pipeline fill (xT is first read ~8us in).
            nc.sync.dma_start(out=s4d[:], in_=d_s4d[:])
            nc.sync.dma_start(out=bias_sb[:], in_=d_bias[:])
            nc.sync.dma_start(out=ident[:], in_=d_ident[:])
            s4d_v = s4d.rearrange("p (g t) -> p g t", t=2)

            GW = 4            # t-columns gathered per transpose/evac tile
            evac_ctr = [0]
            b_off = [0]       # running column offset into the d_b stream

            for ks in range(2):             # k-super: 512 out cols of y
                y_ps = psumy.tile([M, 512], F32, tag="yps")
                nc.tensor.matmul(
                    y_ps[:], lhsT=ones[:],
                    rhs=bias_sb[:, ks * 512:(ks + 1) * 512],
                    start=True, stop=False,
                )
                mm = 0
                # software-pipeline: y-matmuls trail the transposes by one
                # gather tile so evacuation is off the PE critical path
                # (PE executes strictly in program order).
                pending = []

                def flush_mm(limit):
                    nonlocal mm
                    while len(pending) > limit:
                        wT, t0 = pending.pop(0)
                        for tl in range(GW):
                            mm += 1
                            nc.tensor.matmul(
                                y_ps[:],
                                lhsT=xT[:, (t0 + tl) * M:(t0 + tl + 1) * M],
                                rhs=wT[:, tl * 512:(tl + 1) * 512],
                                start=False, stop=(mm == NT),
                            )

                widths = WIDTHS0 if ks == 0 else WIDTHS1
                phases = []
                n0 = 0
                for nw_ in widths:
                    phases.append((n0, nw_))
                    n0 += nw_
                for n0, nw in phases:       # n phases
                    GH = nw // GS
                    w4 = []
                    for k4 in range(4):
                        kt = ks * 4 + k4
                        b_st = stagep.tile([128, nw], F16, tag=f"b{k4}", bufs=2)
                        src = d_b[:, b_off[0]:b_off[0] + nw]
                        b_off[0] += nw
                        # cast-DMAs (int8 -> f16) ride the SWDGE queue
                        nc.gpsimd.dma_start(out=b_st[:], in_=src)
                        w = wpool.tile([128, nw], F16, tag=f"w{k4}", bufs=3)
                        goff = kt * NG + n0 // GS
                        sb = s4d_v[:, goff:goff + GH, :].unsqueeze(2).to_broadcast(
                            [128, GH, 32, 2])
                        b4 = b_st.rearrange("p (g r t) -> p g r t", r=32, t=2)
                        w4v = w.rearrange("p (g r t) -> p g r t", r=32, t=2)
                        # the entire dequant: w = W8 * s4[k, g]
                        nc.vector.tensor_tensor(out=w4v[:], in0=b4[:], in1=sb[:], op=mult)
                        w4.append(w)
                    if ks == 0 and n0 == 0:
                        nc.sync.dma_start(out=xT[:], in_=d_xt[:])
                    TH = nw // 128
                    for tg in range(TH // GW):
                        ps_t = psumt.tile([128, GW * 512], F16, tag="pst")
                        # k4-major: the first transposes of a phase only need
                        # the k4=0 dequant, smoothing pipeline fill
                        for k4 in range(4):
                            for tl in range(GW):
                                t = tg * GW + tl
                                nc.tensor.transpose(
                                    ps_t[:, (tl * 4 + k4) * 128:(tl * 4 + k4 + 1) * 128],
                                    w4[k4][:, t * 128:(t + 1) * 128],
                                    ident[:],
                                )
                        wT = stagep.tile([128, GW * 512], F16, tag="wT", bufs=3)
                        # evacs mostly on ScalarE; every 3rd on the DVE
                        # (which has headroom) to keep ACT off the critical
                        # path
                        if evac_ctr[0] % 3 == 2:
                            nc.vector.tensor_copy(wT[:], ps_t[:])
                        else:
                            nc.scalar.copy(wT[:], ps_t[:])
                        evac_ctr[0] += 1
                        pending.append((wT, n0 // 128 + tg * GW))
                        flush_mm(2)
                flush_mm(0)
                y_sb = stagep.tile([M, 512], F32, tag="ysb")
                nc.scalar.copy(y_sb[:], y_ps[:])
                nc.sync.dma_start(out=d_y[:, ks * 512:(ks + 1) * 512], in_=y_sb[:])

    nc.compile()
    return nc


def _get_compiled():
    global _compiled
    if _compiled is None:
        _compiled = _build()
    return _compiled


def make_in_maps(x, W_q, scales, zeros, mask, mu1, mu2, bias):
    x = np.asarray(x, dtype=np.float32)
    W_q = np.asarray(W_q, dtype=np.float32).reshape(K, N)
    scales = np.asarray(scales, dtype=np.float32).reshape(K, NG)
    zeros = np.asarray(zeros, dtype=np.float32).reshape(K, NG)
    mask_f = np.asarray(mask, dtype=np.float32)
    mu1 = np.asarray(mu1, dtype=np.float32)
    mu2 = np.asarray(mu2, dtype=np.float32)
    bias = np.asarray(bias, dtype=np.float32)

    # symmetric per-group re-encode: W8 = round(127 (q - z)/alpha) * mask,
    # alpha = max|q - z| over the group's kept weights
    qz = (W_q - np.repeat(zeros, GS, axis=1)) * mask_f        # [K, N]
    amax = np.abs(qz).reshape(K, NG, GS).max(axis=2)          # [K, NG]
    amax[amax == 0.0] = 1.0
    W8 = np.rint(qz * np.repeat(127.0 / amax, GS, axis=1)).astype(np.int8)
    sc4 = scales * (amax / 127.0) * mu2[:, None]              # folded scales

    # pre-transposed, mu1-folded x as f16
    xtp = np.ascontiguousarray(
        (x * mu1[None, :]).astype(np.float16).reshape(M, NT, 128)
        .transpose(2, 1, 0)).reshape(128, NT * M)

    # stream-order the W8 slabs: for (ks, phase, k4): [128, nw] with
    # partition p = k-row kt*128+p, columns n0:n0+nw
    def pack_b(b_r):
        bt = b_r.reshape(NKT, 128, N)  # [kt, p, n]
        slabs = []
        for ks, widths in ((0, WIDTHS0), (1, WIDTHS1)):
            n0 = 0
            for nw in widths:
                for k4 in range(4):
                    slabs.append(bt[ks * 4 + k4, :, n0:n0 + nw])
                n0 += nw
        return np.ascontiguousarray(np.concatenate(slabs, axis=1))

    in_maps = []
    for c in range(N_CORES):
        r = slice(c * KL, (c + 1) * KL)
        # s4d[p, (kt, g, t)] = sc4[kt*128+p, g] pair-duplicated along t
        sc_t = sc4[r].reshape(NKT, 128, NG).transpose(1, 0, 2)   # [128, NKT, NG]
        s4d = np.repeat(sc_t.reshape(128, NKT * NG), 2, axis=1).astype(np.float16)
        in_maps.append({
            "b": pack_b(W8[r]),
            "s4d": np.ascontiguousarray(s4d),
            "bias": np.ascontiguousarray(bias[r].reshape(1, KL).astype(np.float16)),
            "xt": xtp,
            "ident": np.eye(128, dtype=np.float16),
        })
    return in_maps


def kernel(x, W_q, scales, zeros, mask, mu1, mu2, bias, **run_kwargs):
    nc = _get_compiled()
    in_maps = make_in_maps(x, W_q, scales, zeros, mask, mu1, mu2, bias)
    res = bass_utils.run_bass_kernel_spmd(
        nc, in_maps, core_ids=list(range(N_CORES)), **run_kwargs
    )
    y = np.concatenate([res.results[c]["y"] for c in range(N_CORES)], axis=1)
    if run_kwargs:
        return y, res
    return y
